# revision 15
# baseline (speedup 1.0000x reference)
"""LoRALinear kernel for Trainium2 (8 NeuronCores, SPMD data-parallel).

Computes out = x @ W.T + b + SCALE*((x@gA.T)@gB.T + (x@lA.T)@lB.T)
  x: [8, 2048, 1024] f32, W: [4096, 1024], b: [4096]
  gA/lA: [8, 1024], gB/lB: [4096, 8]  ->  out: [8, 2048, 4096] f32

Data-parallel: core i handles batch i. Host marshals layouts so the
device does nothing but matmuls and psum evictions:
  - xT   [1024, 2048] fp16: x[i].T  (k on partitions -> no PE transposes)
  - WtT  [8192, 512]  fp16: W.T tiled [ot][kt][128, 512] so o-tile ot is
    one contiguous 1MB chunk (ot-outer pipeline starts after 1MB of DMA)
  - A_cat = SCALE*[gA;lA] [16, 1024], B_catT = [gB.T;lB.T] [16, 4096]

Device, per o-tile (512 cols), software-pipelined one ahead:
  build W_eff chunk: DMA W.T chunk + rank-16 LoRA matmul into f32 psum,
  DVE-added in place (fp16).  Then 16 s-tiles x 8 k-tile fp16 matmuls
  accumulate into f32 psum; DVE adds bias (PE-broadcast once) and writes
  fp16 out tile; DMA to DRAM. Host casts fp16 out back to f32.

All-fp16 PE ops keep LDWEIGHTS pipelined: main GEMM streams at
512 cols/matmul back-to-back = the 78.6 TF/s fp16 roofline.
fp16 in/out rounding gives ~8e-4 absmax rel err (f32 psum accumulate).
"""
import numpy as np
from contextlib import ExitStack

import concourse.bass as bass
import concourse.tile as tile
from concourse import bacc, mybir
from concourse.bass import ts, ds
from concourse.bass_utils import run_bass_kernel_spmd

F32 = mybir.dt.float32
F16 = mybir.dt.float16
F8 = mybir.dt.float8e4

# LoRA factors go to the PE as fp8 DoubleRow (0.5 cycles/row). Rank-48
# error-corrected stack [A8; dA8; A8] x [B8; B8; dB8] cancels first-order
# fp8 quantization error (~1e-4 rel); one common product scale SA*SB is
# divided out during the psum eviction. Scales keep residuals normal-range.
SA, SB = 16.0, 128.0
R48, RP = 48, 24  # stacked rank, DoubleRow partition count (48 = 24 x 2)

N_CORES = 8
B, S, DIN, DOUT, R = 8, 2048, 1024, 4096, 8
SCALE = 16.0 / 8
R2 = 2 * R

P = 128            # partition tile
OTILE = 512        # matmul moving free dim (one PSUM bank of f32)
KT = DIN // P      # 8 k-tiles
OT = DOUT // OTILE # 8 o-tiles
ST = S // P        # 16 s-tiles


def build_nc():
    nc = bacc.Bacc("TRN2", target_bir_lowering=False, debug=False,
                   num_devices=N_CORES)
    xT = nc.dram_tensor("xT", [DIN, S], F16, kind="ExternalInput").ap()
    WtT = nc.dram_tensor("WtT", [OT * KT * P, OTILE], F16,
                         kind="ExternalInput").ap()
    bvec = nc.dram_tensor("b16", [DOUT], F16, kind="ExternalInput").ap()
    A48 = nc.dram_tensor("A48", [RP, 2, DIN], F8, kind="ExternalInput").ap()
    B48 = nc.dram_tensor("B48", [RP, 2, DOUT], F8,
                         kind="ExternalInput").ap()
    out = nc.dram_tensor("out", [S, DOUT], F16, kind="ExternalOutput").ap()

    with tile.TileContext(nc) as tc:
        with ExitStack() as ctx:
            const = ctx.enter_context(tc.tile_pool(name="const", bufs=1))
            xt_pool = ctx.enter_context(tc.tile_pool(name="xt", bufs=1))
            wet_pool = ctx.enter_context(tc.tile_pool(name="wet", bufs=3))
            out_pool = ctx.enter_context(tc.tile_pool(name="outp", bufs=4))
            ps_aux = ctx.enter_context(
                tc.tile_pool(name="psaux", bufs=4, space="PSUM"))
            ps_main = ctx.enter_context(
                tc.tile_pool(name="psmain", bufs=4, space="PSUM"))

            # consts; sync queue gets [acat, bcatt, wet...] triggers,
            # scalar queue gets [brow, xt..., out...] triggers (each
            # dma_start costs ~600ns serialized on its trigger queue)
            ones_col = const.tile([1, P], F16)
            nc.vector.memset(ones_col[:], 1.0)
            acat = const.tile([RP, 2, DIN], F8)
            nc.sync.dma_start(acat[:], A48)
            bcatt = const.tile([RP, 2, DOUT], F8)
            nc.sync.dma_start(bcatt[:], B48)
            brow16 = const.tile([1, DOUT], F16)
            nc.scalar.dma_start(brow16[:], bvec[None, :])
            bias_sb = const.tile([P, DOUT], F32)

            # PE p-state warmup: the PE clock sits at half speed until the
            # DVFS governor steps it up (~43us in). Full-array junk matmuls
            # present maximum switching activity to the governor while the
            # first DMAs land.
            junk = const.tile([P, P + OTILE], F16)
            nc.gpsimd.memset(junk[:], 1.0)
            for i in range(44):
                pw = ps_main.tile([P, OTILE], F32, tag="psmain")
                nc.tensor.matmul(pw[:], junk[:, :P], junk[:, P:],
                                 start=True, stop=True)

            # W_eff chunks, triple-buffered per kt tag: [128 k, 512 o] fp16.
            # Each build also broadcasts its bias chunk (rank-1 matmul).
            wet = [[None] * KT for _ in range(OT)]

            def build_wet(ot):
                for kt in range(KT):
                    w = wet_pool.tile([P, OTILE], F16, tag=f"wet{kt}",
                                      name=f"wet{ot}_{kt}")
                    nc.sync.dma_start(
                        w[:], WtT[ds((ot * KT + kt) * P, P), :])
                    wet[ot][kt] = w
                for kt in range(KT):
                    pl = ps_aux.tile([P, OTILE], F32, tag="psaux")
                    nc.tensor.matmul(pl[:], acat[:, :, ts(kt, P)],
                                     bcatt[:, :, ts(ot, OTILE)],
                                     start=True, stop=True,
                                     perf_mode=mybir.MatmulPerfMode.DoubleRow)
                    w = wet[ot][kt]
                    nc.vector.scalar_tensor_tensor(
                        w[:], pl[:], 1.0 / (SA * SB), w[:],
                        mybir.AluOpType.mult, mybir.AluOpType.add)
                pb = ps_aux.tile([P, OTILE], F32, tag="psaux")
                nc.tensor.matmul(pb[:], ones_col[:],
                                 brow16[:, ts(ot, OTILE)],
                                 start=True, stop=True)
                nc.vector.tensor_copy(bias_sb[:, ts(ot, OTILE)], pb[:])

            build_wet(0)

            # resident x.T: 8 tiles [128 k, 2048 s], 4KB/partition
            xt = []
            for kt in range(KT):
                t = xt_pool.tile([P, S], F16, tag=f"xt{kt}", name=f"xt{kt}")
                nc.scalar.dma_start(t[:], xT[ts(kt, P), :])
                xt.append(t)

            build_wet(1)

            # ---- main: ot-outer, build W_eff[ot+1] ahead of s-loop[ot] ----
            for ot in range(OT):
                if 2 <= ot + 1 < OT:
                    build_wet(ot + 1)
                for st in range(ST):
                    po = ps_main.tile([P, OTILE], F32, tag="psmain")
                    for kt in range(KT):
                        nc.tensor.matmul(po[:], xt[kt][:, ts(st, P)],
                                         wet[ot][kt][:],
                                         start=(kt == 0), stop=(kt == KT - 1))
                    osb = out_pool.tile([P, OTILE], F16)
                    nc.vector.tensor_tensor(osb[:], po[:],
                                            bias_sb[:, ts(ot, OTILE)],
                                            mybir.AluOpType.add)
                    nc.scalar.dma_start(out[ts(st, P), ts(ot, OTILE)], osb[:])

    nc.compile()
    return nc


_NC_CACHE = None


def _get_nc():
    global _NC_CACHE
    if _NC_CACHE is None:
        _NC_CACHE = build_nc()
    return _NC_CACHE


def make_in_maps(x, W, b, global_A, global_B, local_A, local_B):
    import ml_dtypes
    F8NP = ml_dtypes.float8_e4m3

    x = np.asarray(x, dtype=np.float32)
    W = np.asarray(W, dtype=np.float32)
    b = np.asarray(b, dtype=np.float32)
    # W.T tiled [ot][kt][128, 512] -> [8192, 512] so each o-tile is contiguous
    WtT = np.ascontiguousarray(
        W.T.reshape(KT, P, OT, OTILE).transpose(2, 0, 1, 3)
    ).reshape(OT * KT * P, OTILE).astype(np.float16)

    A = SCALE * np.concatenate([np.asarray(global_A), np.asarray(local_A)],
                               axis=0).astype(np.float32)   # [16, DIN]
    B = np.concatenate([np.asarray(global_B).T, np.asarray(local_B).T],
                       axis=0).astype(np.float32)           # [16, DOUT]
    A8 = (A * SA).astype(F8NP)
    dA8 = (A * SA - A8.astype(np.float32)).astype(F8NP)
    B8 = (B * SB).astype(F8NP)
    dB8 = (B * SB - B8.astype(np.float32)).astype(F8NP)
    A48 = np.concatenate([A8, dA8, A8], axis=0)             # [48, DIN]
    B48 = np.concatenate([B8, B8, dB8], axis=0)             # [48, DOUT]
    # DoubleRow pairing [24, 2, D]: row r = i*24 + p
    A48dr = np.ascontiguousarray(A48.reshape(2, RP, DIN).transpose(1, 0, 2))
    B48dr = np.ascontiguousarray(B48.reshape(2, RP, DOUT).transpose(1, 0, 2))

    b16 = b.astype(np.float16)
    return [
        {"xT": np.ascontiguousarray(x[i].T).astype(np.float16),
         "WtT": WtT, "b16": b16, "A48": A48dr, "B48": B48dr}
        for i in range(N_CORES)
    ]


def kernel(x, W, b, global_A, global_B, local_A, local_B):
    nc = _get_nc()
    in_maps = make_in_maps(x, W, b, global_A, global_B, local_A, local_B)
    res = run_bass_kernel_spmd(nc, in_maps, list(range(N_CORES))).results
    return np.stack([res[i]["out"].astype(np.float32)
                     for i in range(N_CORES)], axis=0)


# revision 16
# speedup vs baseline: 1.0257x; 1.0257x over previous
"""LoRALinear kernel for Trainium2 (8 NeuronCores, SPMD data-parallel).

Computes out = x @ W.T + b + SCALE*((x@gA.T)@gB.T + (x@lA.T)@lB.T)
  x: [8, 2048, 1024] f32, W: [4096, 1024], b: [4096]
  gA/lA: [8, 1024], gB/lB: [4096, 8]  ->  out: [8, 2048, 4096] f32

Data-parallel: core i handles batch i. Host marshals layouts so the
device does nothing but matmuls and psum evictions:
  - xT   [1024, 2048] fp16: x[i].T  (k on partitions -> no PE transposes)
  - WtT  [8192, 512]  fp16: W.T tiled [ot][kt][128, 512] so o-tile ot is
    one contiguous 1MB chunk (ot-outer pipeline starts after 1MB of DMA)
  - A_cat = SCALE*[gA;lA] [16, 1024], B_catT = [gB.T;lB.T] [16, 4096]

Device, per o-tile (512 cols), software-pipelined one ahead:
  build W_eff chunk: DMA W.T chunk + rank-16 LoRA matmul into f32 psum,
  DVE-added in place (fp16).  Then 16 s-tiles x 8 k-tile fp16 matmuls
  accumulate into f32 psum; DVE adds bias (PE-broadcast once) and writes
  fp16 out tile; DMA to DRAM. Host casts fp16 out back to f32.

All-fp16 PE ops keep LDWEIGHTS pipelined: main GEMM streams at
512 cols/matmul back-to-back = the 78.6 TF/s fp16 roofline.
fp16 in/out rounding gives ~8e-4 absmax rel err (f32 psum accumulate).
"""
import numpy as np
from contextlib import ExitStack

import concourse.bass as bass
import concourse.tile as tile
from concourse import bacc, mybir
from concourse.bass import ts, ds
from concourse.bass_utils import run_bass_kernel_spmd

F32 = mybir.dt.float32
F16 = mybir.dt.float16
F8 = mybir.dt.float8e4

# LoRA factors go to the PE as fp8 DoubleRow (0.5 cycles/row). Rank-48
# error-corrected stack [A8; dA8; A8] x [B8; B8; dB8] cancels first-order
# fp8 quantization error (~1e-4 rel); one common product scale SA*SB is
# divided out during the psum eviction. Scales keep residuals normal-range.
SA, SB = 16.0, 128.0
R48, RP = 48, 24  # stacked rank, DoubleRow partition count (48 = 24 x 2)

N_CORES = 8
B, S, DIN, DOUT, R = 8, 2048, 1024, 4096, 8
SCALE = 16.0 / 8
R2 = 2 * R

P = 128            # partition tile
OTILE = 512        # matmul moving free dim (one PSUM bank of f32)
KT = DIN // P      # 8 k-tiles
OT = DOUT // OTILE # 8 o-tiles
ST = S // P        # 16 s-tiles


def build_nc():
    nc = bacc.Bacc("TRN2", target_bir_lowering=False, debug=False,
                   num_devices=N_CORES)
    xT = nc.dram_tensor("xT", [DIN, S], F16, kind="ExternalInput").ap()
    WtT = nc.dram_tensor("WtT", [OT * KT * P, OTILE], F16,
                         kind="ExternalInput").ap()
    bvec = nc.dram_tensor("b16", [DOUT], F16, kind="ExternalInput").ap()
    A48 = nc.dram_tensor("A48", [RP, 2, DIN], F8, kind="ExternalInput").ap()
    B48 = nc.dram_tensor("B48", [RP, 2, DOUT], F8,
                         kind="ExternalInput").ap()
    out = nc.dram_tensor("out", [S, DOUT], F16, kind="ExternalOutput").ap()

    with tile.TileContext(nc) as tc:
        with ExitStack() as ctx:
            const = ctx.enter_context(tc.tile_pool(name="const", bufs=1))
            xt_pool = ctx.enter_context(tc.tile_pool(name="xt", bufs=1))
            wet_pool = ctx.enter_context(tc.tile_pool(name="wet", bufs=3))
            out_pool = ctx.enter_context(tc.tile_pool(name="outp", bufs=4))
            ps_aux = ctx.enter_context(
                tc.tile_pool(name="psaux", bufs=4, space="PSUM"))
            ps_main = ctx.enter_context(
                tc.tile_pool(name="psmain", bufs=4, space="PSUM"))

            # consts; sync queue gets [acat, bcatt, wet...] triggers,
            # scalar queue gets [brow, xt..., out...] triggers (each
            # dma_start costs ~600ns serialized on its trigger queue)
            ones_col = const.tile([1, P], F16)
            nc.vector.memset(ones_col[:], 1.0)
            acat = const.tile([RP, 2, DIN], F8)
            nc.sync.dma_start(acat[:], A48)
            bcatt = const.tile([RP, 2, DOUT], F8)
            nc.sync.dma_start(bcatt[:], B48)
            brow16 = const.tile([1, DOUT], F16)
            nc.scalar.dma_start(brow16[:], bvec[None, :])
            bias_sb = const.tile([P, DOUT], F32)

            # PE pipeline warmup: near-zero-power dummy matmuls lift the PE
            # pipe off its lowest p-state while the first DMAs land. Heavier
            # warmup is counterproductive: the DVFS governor repays any early
            # k=8 boost by delaying the sustained full-clock step ~1:1.
            for i in range(32):
                pw = ps_main.tile([P, OTILE], F32, tag="psmain")
                nc.tensor.matmul(pw[:, :P], ones_col[:], ones_col[:],
                                 start=True, stop=True)

            # W_eff chunks, triple-buffered per kt tag: [128 k, 512 o] fp16.
            # Each build also broadcasts its bias chunk (rank-1 matmul).
            wet = [[None] * KT for _ in range(OT)]

            def build_wet(ot):
                for kt in range(KT):
                    w = wet_pool.tile([P, OTILE], F16, tag=f"wet{kt}",
                                      name=f"wet{ot}_{kt}")
                    nc.sync.dma_start(
                        w[:], WtT[ds((ot * KT + kt) * P, P), :])
                    wet[ot][kt] = w
                for kt in range(KT):
                    pl = ps_aux.tile([P, OTILE], F32, tag="psaux")
                    nc.tensor.matmul(pl[:], acat[:, :, ts(kt, P)],
                                     bcatt[:, :, ts(ot, OTILE)],
                                     start=True, stop=True,
                                     perf_mode=mybir.MatmulPerfMode.DoubleRow)
                    w = wet[ot][kt]
                    nc.vector.scalar_tensor_tensor(
                        w[:], pl[:], 1.0 / (SA * SB), w[:],
                        mybir.AluOpType.mult, mybir.AluOpType.add)
                pb = ps_aux.tile([P, OTILE], F32, tag="psaux")
                nc.tensor.matmul(pb[:], ones_col[:],
                                 brow16[:, ts(ot, OTILE)],
                                 start=True, stop=True)
                nc.vector.tensor_copy(bias_sb[:, ts(ot, OTILE)], pb[:])

            build_wet(0)

            # resident x.T: 8 tiles [128 k, 2048 s], 4KB/partition
            xt = []
            for kt in range(KT):
                t = xt_pool.tile([P, S], F16, tag=f"xt{kt}", name=f"xt{kt}")
                nc.scalar.dma_start(t[:], xT[ts(kt, P), :])
                xt.append(t)

            build_wet(1)

            # ---- main: ot-outer, build W_eff[ot+1] ahead of s-loop[ot] ----
            for ot in range(OT):
                if 2 <= ot + 1 < OT:
                    build_wet(ot + 1)
                for st in range(ST):
                    po = ps_main.tile([P, OTILE], F32, tag="psmain")
                    for kt in range(KT):
                        nc.tensor.matmul(po[:], xt[kt][:, ts(st, P)],
                                         wet[ot][kt][:],
                                         start=(kt == 0), stop=(kt == KT - 1))
                    osb = out_pool.tile([P, OTILE], F16)
                    nc.vector.tensor_tensor(osb[:], po[:],
                                            bias_sb[:, ts(ot, OTILE)],
                                            mybir.AluOpType.add)
                    nc.scalar.dma_start(out[ts(st, P), ts(ot, OTILE)], osb[:])

    nc.compile()
    return nc


_NC_CACHE = None


def _get_nc():
    global _NC_CACHE
    if _NC_CACHE is None:
        _NC_CACHE = build_nc()
    return _NC_CACHE


def make_in_maps(x, W, b, global_A, global_B, local_A, local_B):
    import ml_dtypes
    F8NP = ml_dtypes.float8_e4m3

    x = np.asarray(x, dtype=np.float32)
    W = np.asarray(W, dtype=np.float32)
    b = np.asarray(b, dtype=np.float32)
    # W.T tiled [ot][kt][128, 512] -> [8192, 512] so each o-tile is contiguous
    WtT = np.ascontiguousarray(
        W.T.reshape(KT, P, OT, OTILE).transpose(2, 0, 1, 3)
    ).reshape(OT * KT * P, OTILE).astype(np.float16)

    A = SCALE * np.concatenate([np.asarray(global_A), np.asarray(local_A)],
                               axis=0).astype(np.float32)   # [16, DIN]
    B = np.concatenate([np.asarray(global_B).T, np.asarray(local_B).T],
                       axis=0).astype(np.float32)           # [16, DOUT]
    A8 = (A * SA).astype(F8NP)
    dA8 = (A * SA - A8.astype(np.float32)).astype(F8NP)
    B8 = (B * SB).astype(F8NP)
    dB8 = (B * SB - B8.astype(np.float32)).astype(F8NP)
    A48 = np.concatenate([A8, dA8, A8], axis=0)             # [48, DIN]
    B48 = np.concatenate([B8, B8, dB8], axis=0)             # [48, DOUT]
    # DoubleRow pairing [24, 2, D]: row r = i*24 + p
    A48dr = np.ascontiguousarray(A48.reshape(2, RP, DIN).transpose(1, 0, 2))
    B48dr = np.ascontiguousarray(B48.reshape(2, RP, DOUT).transpose(1, 0, 2))

    b16 = b.astype(np.float16)
    return [
        {"xT": np.ascontiguousarray(x[i].T).astype(np.float16),
         "WtT": WtT, "b16": b16, "A48": A48dr, "B48": B48dr}
        for i in range(N_CORES)
    ]


def kernel(x, W, b, global_A, global_B, local_A, local_B):
    nc = _get_nc()
    in_maps = make_in_maps(x, W, b, global_A, global_B, local_A, local_B)
    res = run_bass_kernel_spmd(nc, in_maps, list(range(N_CORES))).results
    return np.stack([res[i]["out"].astype(np.float32)
                     for i in range(N_CORES)], axis=0)


# revision 17
# speedup vs baseline: 1.0375x; 1.0115x over previous
"""LoRALinear kernel for Trainium2 (8 NeuronCores, SPMD data-parallel).

Computes out = x @ W.T + b + SCALE*((x@gA.T)@gB.T + (x@lA.T)@lB.T)
  x: [8, 2048, 1024] f32, W: [4096, 1024], b: [4096]
  gA/lA: [8, 1024], gB/lB: [4096, 8]  ->  out: [8, 2048, 4096] f32

Data-parallel: core i handles batch i. Host marshals layouts so the
device does nothing but matmuls and psum evictions:
  - xT   [1024, 2048] fp16: x[i].T  (k on partitions -> no PE transposes)
  - WtT  [8192, 512]  fp16: W.T tiled [ot][kt][128, 512] so o-tile ot is
    one contiguous 1MB chunk (ot-outer pipeline starts after 1MB of DMA)
  - A_cat = SCALE*[gA;lA] [16, 1024], B_catT = [gB.T;lB.T] [16, 4096]

Device, per o-tile (512 cols), software-pipelined one ahead:
  build W_eff chunk: DMA W.T chunk + rank-16 LoRA matmul into f32 psum,
  DVE-added in place (fp16).  Then 16 s-tiles x 8 k-tile fp16 matmuls
  accumulate into f32 psum; DVE adds bias (PE-broadcast once) and writes
  fp16 out tile; DMA to DRAM. Host casts fp16 out back to f32.

All-fp16 PE ops keep LDWEIGHTS pipelined: main GEMM streams at
512 cols/matmul back-to-back = the 78.6 TF/s fp16 roofline.
fp16 in/out rounding gives ~8e-4 absmax rel err (f32 psum accumulate).
"""
import numpy as np
from contextlib import ExitStack

import concourse.bass as bass
import concourse.tile as tile
from concourse import bacc, mybir
from concourse.bass import ts, ds
from concourse.bass_utils import run_bass_kernel_spmd

F32 = mybir.dt.float32
F16 = mybir.dt.float16

N_CORES = 8
B, S, DIN, DOUT, R = 8, 2048, 1024, 4096, 8
SCALE = 16.0 / 8
R2 = 2 * R

P = 128            # partition tile
OTILE = 512        # matmul moving free dim (one PSUM bank of f32)
KT = DIN // P      # 8 k-tiles
OT = DOUT // OTILE # 8 o-tiles
ST = S // P        # 16 s-tiles


def build_nc():
    nc = bacc.Bacc("TRN2", target_bir_lowering=False, debug=False,
                   num_devices=N_CORES)
    xT = nc.dram_tensor("xT", [DIN, S], F16, kind="ExternalInput").ap()
    WtT = nc.dram_tensor("WtT", [OT * KT * P, OTILE], F16,
                         kind="ExternalInput").ap()
    bvec = nc.dram_tensor("b16", [DOUT], F16, kind="ExternalInput").ap()
    A_cat = nc.dram_tensor("A_cat", [R2, DIN], F16, kind="ExternalInput").ap()
    B_catT = nc.dram_tensor("B_catT", [R2, DOUT], F16,
                            kind="ExternalInput").ap()
    out = nc.dram_tensor("out", [S, DOUT], F16, kind="ExternalOutput").ap()

    with tile.TileContext(nc) as tc:
        with ExitStack() as ctx:
            const = ctx.enter_context(tc.tile_pool(name="const", bufs=1))
            xt_pool = ctx.enter_context(tc.tile_pool(name="xt", bufs=1))
            wet_pool = ctx.enter_context(tc.tile_pool(name="wet", bufs=3))
            out_pool = ctx.enter_context(tc.tile_pool(name="outp", bufs=4))
            ps_aux = ctx.enter_context(
                tc.tile_pool(name="psaux", bufs=4, space="PSUM"))
            ps_main = ctx.enter_context(
                tc.tile_pool(name="psmain", bufs=4, space="PSUM"))

            # consts; sync queue gets [acat, bcatt, wet...] triggers,
            # scalar queue gets [brow, xt..., out...] triggers (each
            # dma_start costs ~600ns serialized on its trigger queue)
            ones_col = const.tile([1, P], F16)
            nc.vector.memset(ones_col[:], 1.0)
            acat = const.tile([R2, DIN], F16)
            nc.sync.dma_start(acat[:], A_cat)
            bcatt = const.tile([R2, DOUT], F16)
            nc.sync.dma_start(bcatt[:], B_catT)
            brow16 = const.tile([1, DOUT], F16)
            nc.scalar.dma_start(brow16[:], bvec[None, :])
            bias_sb = const.tile([P, DOUT], F32)

            # PE pipeline warmup: near-zero-power dummy matmuls lift the PE
            # pipe off its lowest p-state while the first DMAs land. Heavier
            # warmup is counterproductive: the DVFS governor repays any early
            # k=8 boost by delaying the sustained full-clock step ~1:1.
            for i in range(32):
                pw = ps_main.tile([P, OTILE], F32, tag="psmain")
                nc.tensor.matmul(pw[:, :P], ones_col[:], ones_col[:],
                                 start=True, stop=True)

            # W_eff chunks, triple-buffered per kt tag: [128 k, 512 o] fp16.
            # Each build also broadcasts its bias chunk (rank-1 matmul).
            wet = [[None] * KT for _ in range(OT)]

            def build_wet(ot):
                for kt in range(KT):
                    w = wet_pool.tile([P, OTILE], F16, tag=f"wet{kt}",
                                      name=f"wet{ot}_{kt}")
                    nc.sync.dma_start(
                        w[:], WtT[ds((ot * KT + kt) * P, P), :])
                    wet[ot][kt] = w
                for kt in range(KT):
                    pl = ps_aux.tile([P, OTILE], F32, tag="psaux")
                    nc.tensor.matmul(pl[:], acat[:, ts(kt, P)],
                                     bcatt[:, ts(ot, OTILE)],
                                     start=True, stop=True)
                    w = wet[ot][kt]
                    nc.vector.tensor_tensor(w[:], pl[:], w[:],
                                            mybir.AluOpType.add)
                pb = ps_aux.tile([P, OTILE], F32, tag="psaux")
                nc.tensor.matmul(pb[:], ones_col[:],
                                 brow16[:, ts(ot, OTILE)],
                                 start=True, stop=True)
                nc.vector.tensor_copy(bias_sb[:, ts(ot, OTILE)], pb[:])

            build_wet(0)

            # resident x.T: 8 tiles [128 k, 2048 s], 4KB/partition
            xt = []
            for kt in range(KT):
                t = xt_pool.tile([P, S], F16, tag=f"xt{kt}", name=f"xt{kt}")
                nc.scalar.dma_start(t[:], xT[ts(kt, P), :])
                xt.append(t)

            build_wet(1)

            # ---- main: ot-outer, build W_eff[ot+1] ahead of s-loop[ot] ----
            for ot in range(OT):
                if 2 <= ot + 1 < OT:
                    build_wet(ot + 1)
                for st in range(ST):
                    po = ps_main.tile([P, OTILE], F32, tag="psmain")
                    for kt in range(KT):
                        nc.tensor.matmul(po[:], xt[kt][:, ts(st, P)],
                                         wet[ot][kt][:],
                                         start=(kt == 0), stop=(kt == KT - 1))
                    osb = out_pool.tile([P, OTILE], F16)
                    nc.vector.tensor_tensor(osb[:], po[:],
                                            bias_sb[:, ts(ot, OTILE)],
                                            mybir.AluOpType.add)
                    nc.scalar.dma_start(out[ts(st, P), ts(ot, OTILE)], osb[:])

    nc.compile()
    return nc


_NC_CACHE = None


def _get_nc():
    global _NC_CACHE
    if _NC_CACHE is None:
        _NC_CACHE = build_nc()
    return _NC_CACHE


def make_in_maps(x, W, b, global_A, global_B, local_A, local_B):
    x = np.asarray(x, dtype=np.float32)
    W = np.asarray(W, dtype=np.float32)
    b = np.asarray(b, dtype=np.float32)
    # W.T tiled [ot][kt][128, 512] -> [8192, 512] so each o-tile is contiguous
    WtT = np.ascontiguousarray(
        W.T.reshape(KT, P, OT, OTILE).transpose(2, 0, 1, 3)
    ).reshape(OT * KT * P, OTILE).astype(np.float16)

    A_cat = np.ascontiguousarray(
        SCALE * np.concatenate([np.asarray(global_A), np.asarray(local_A)],
                               axis=0)).astype(np.float16)
    B_catT = np.ascontiguousarray(
        np.concatenate([np.asarray(global_B).T, np.asarray(local_B).T],
                       axis=0)).astype(np.float16)
    b16 = b.astype(np.float16)
    return [
        {"xT": np.ascontiguousarray(x[i].T).astype(np.float16),
         "WtT": WtT, "b16": b16, "A_cat": A_cat, "B_catT": B_catT}
        for i in range(N_CORES)
    ]


def kernel(x, W, b, global_A, global_B, local_A, local_B):
    nc = _get_nc()
    in_maps = make_in_maps(x, W, b, global_A, global_B, local_A, local_B)
    res = run_bass_kernel_spmd(nc, in_maps, list(range(N_CORES))).results
    return np.stack([res[i]["out"].astype(np.float32)
                     for i in range(N_CORES)], axis=0)


# revision 18
# speedup vs baseline: 1.1468x; 1.1054x over previous
"""LoRALinear kernel for Trainium2 (8 NeuronCores, SPMD data-parallel).

Computes out = x @ W.T + b + SCALE*((x@gA.T)@gB.T + (x@lA.T)@lB.T)
  x: [8, 2048, 1024] f32, W: [4096, 1024], b: [4096]
  gA/lA: [8, 1024], gB/lB: [4096, 8]  ->  out: [8, 2048, 4096] f32

Data-parallel: core i handles batch i. Host marshals layouts so the
device does nothing but matmuls and psum evictions:
  - xT16 [768, 2048] fp16: x[i].T k-tiles 0-5 (k on partitions, no PE
    transposes); X8 [128, 2, 2048] fp8: k-tiles 6-7 DoubleRow-paired
  - WtT  [8192, 512] fp16: W.T tiled [ot][kt][128, 512] so o-tile ot is
    one contiguous chunk (ot-outer pipeline starts after 1MB of DMA)
  - A_cat = SCALE*[gA;lA] [16, 1024], B_catT = [gB.T;lB.T] [16, 4096]

Device, per o-tile (512 cols), software-pipelined one ahead: build
W_eff chunks (DMA W.T chunk + rank-16 LoRA matmul into f32 psum,
DVE-added in place, fp16; k-tiles 6-7 also DVE-cast to a paired fp8
tile). Each s-tile group accumulates 6 fp16 matmuls (k 0-767) plus one
fp8 DoubleRow matmul (k 768-1023, two k-tiles in one 512-cycle pass —
the moving pair streams 2 rows/cycle) into f32 psum; DVE adds bias
(PE-broadcast once) and writes the fp16 out tile. Host casts to f32.

7 matmuls/group instead of 8 cuts the PE stream ~12%. The fp8 leg
raises absmax rel err to ~1.5e-2 (measured, fixed seed), under the
2e-2 gate; accumulation stays f32 in PSUM.
"""
import numpy as np
from contextlib import ExitStack

import concourse.bass as bass
import concourse.tile as tile
from concourse import bacc, mybir
from concourse.bass import ts, ds
from concourse.bass_utils import run_bass_kernel_spmd

F32 = mybir.dt.float32
F16 = mybir.dt.float16
F8 = mybir.dt.float8e4

N_CORES = 8
B, S, DIN, DOUT, R = 8, 2048, 1024, 4096, 8
SCALE = 16.0 / 8
R2 = 2 * R

P = 128            # partition tile
OTILE = 512        # matmul moving free dim (one PSUM bank of f32)
KT = DIN // P      # 8 k-tiles
KT16 = 6           # k-tiles 0-5 in fp16; 6-7 ride one fp8 DoubleRow matmul
OT = DOUT // OTILE # 8 o-tiles
ST = S // P        # 16 s-tiles


def build_nc():
    nc = bacc.Bacc("TRN2", target_bir_lowering=False, debug=False,
                   num_devices=N_CORES)
    xT16 = nc.dram_tensor("xT16", [KT16 * P, S], F16,
                          kind="ExternalInput").ap()
    X8 = nc.dram_tensor("X8", [P, 2, S], F8, kind="ExternalInput").ap()
    WtT = nc.dram_tensor("WtT", [OT * KT * P, OTILE], F16,
                         kind="ExternalInput").ap()
    bvec = nc.dram_tensor("b16", [DOUT], F16, kind="ExternalInput").ap()
    A_cat = nc.dram_tensor("A_cat", [R2, DIN], F16, kind="ExternalInput").ap()
    B_catT = nc.dram_tensor("B_catT", [R2, DOUT], F16,
                            kind="ExternalInput").ap()
    out = nc.dram_tensor("out", [S, DOUT], F16, kind="ExternalOutput").ap()

    with tile.TileContext(nc) as tc:
        with ExitStack() as ctx:
            const = ctx.enter_context(tc.tile_pool(name="const", bufs=1))
            xt_pool = ctx.enter_context(tc.tile_pool(name="xt", bufs=1))
            wet_pool = ctx.enter_context(tc.tile_pool(name="wet", bufs=3))
            out_pool = ctx.enter_context(tc.tile_pool(name="outp", bufs=4))
            ps_aux = ctx.enter_context(
                tc.tile_pool(name="psaux", bufs=4, space="PSUM"))
            ps_main = ctx.enter_context(
                tc.tile_pool(name="psmain", bufs=4, space="PSUM"))

            # consts; sync queue gets [acat, bcatt, wet...] triggers,
            # scalar queue gets [brow, x..., out...] triggers (each
            # dma_start costs ~600ns serialized on its trigger queue)
            ones_col = const.tile([1, P], F16)
            nc.vector.memset(ones_col[:], 1.0)
            acat = const.tile([R2, DIN], F16)
            nc.sync.dma_start(acat[:], A_cat)
            bcatt = const.tile([R2, DOUT], F16)
            nc.sync.dma_start(bcatt[:], B_catT)
            brow16 = const.tile([1, DOUT], F16)
            nc.scalar.dma_start(brow16[:], bvec[None, :])
            bias_sb = const.tile([P, DOUT], F32)

            # PE pipeline warmup: near-zero-power dummy matmuls lift the PE
            # pipe off its lowest p-state while the first DMAs land. Heavier
            # warmup is counterproductive: the DVFS governor repays any early
            # k=8 boost by delaying the sustained full-clock step ~1:1.
            for i in range(32):
                pw = ps_main.tile([P, OTILE], F32, tag="psmain")
                nc.tensor.matmul(pw[:, :P], ones_col[:], ones_col[:],
                                 start=True, stop=True)

            # W_eff chunks, triple-buffered per kt tag: fp16 [128 k, 512 o];
            # k-tiles 6-7 additionally packed to fp8 [128, 2, 512] for the
            # DoubleRow leg. Each build also broadcasts its bias chunk.
            wet = [[None] * KT for _ in range(OT)]
            w8 = [None] * OT

            def build_wet(ot):
                for kt in range(KT):
                    w = wet_pool.tile([P, OTILE], F16, tag=f"wet{kt}",
                                      name=f"wet{ot}_{kt}")
                    nc.sync.dma_start(
                        w[:], WtT[ds((ot * KT + kt) * P, P), :])
                    wet[ot][kt] = w
                for kt in range(KT):
                    pl = ps_aux.tile([P, OTILE], F32, tag="psaux")
                    nc.tensor.matmul(pl[:], acat[:, ts(kt, P)],
                                     bcatt[:, ts(ot, OTILE)],
                                     start=True, stop=True)
                    w = wet[ot][kt]
                    nc.vector.tensor_tensor(w[:], pl[:], w[:],
                                            mybir.AluOpType.add)
                w8t = wet_pool.tile([P, 2, OTILE], F8, tag="w8",
                                    name=f"w8_{ot}")
                for j in range(2):
                    nc.vector.tensor_copy(w8t[:, j, :], wet[ot][KT16 + j][:])
                w8[ot] = w8t
                pb = ps_aux.tile([P, OTILE], F32, tag="psaux")
                nc.tensor.matmul(pb[:], ones_col[:],
                                 brow16[:, ts(ot, OTILE)],
                                 start=True, stop=True)
                nc.vector.tensor_copy(bias_sb[:, ts(ot, OTILE)], pb[:])

            build_wet(0)

            # resident x.T: 6 fp16 tiles [128 k, 2048 s] + fp8 pair tile
            xt = []
            for kt in range(KT16):
                t = xt_pool.tile([P, S], F16, tag=f"xt{kt}", name=f"xt{kt}")
                nc.scalar.dma_start(t[:], xT16[ts(kt, P), :])
                xt.append(t)
            x8t = xt_pool.tile([P, 2, S], F8, tag="x8t", name="x8t")
            nc.scalar.dma_start(x8t[:], X8)

            build_wet(1)

            # ---- main: ot-outer, build W_eff[ot+1] ahead of s-loop[ot] ----
            for ot in range(OT):
                if 2 <= ot + 1 < OT:
                    build_wet(ot + 1)
                for st in range(ST):
                    po = ps_main.tile([P, OTILE], F32, tag="psmain")
                    for kt in range(KT16):
                        nc.tensor.matmul(po[:], xt[kt][:, ts(st, P)],
                                         wet[ot][kt][:],
                                         start=(kt == 0), stop=False)
                    nc.tensor.matmul(po[:], x8t[:, :, ts(st, P)], w8[ot][:],
                                     start=False, stop=True,
                                     perf_mode=mybir.MatmulPerfMode.DoubleRow)
                    osb = out_pool.tile([P, OTILE], F16)
                    nc.vector.tensor_tensor(osb[:], po[:],
                                            bias_sb[:, ts(ot, OTILE)],
                                            mybir.AluOpType.add)
                    nc.scalar.dma_start(out[ts(st, P), ts(ot, OTILE)], osb[:])

    nc.compile()
    return nc


_NC_CACHE = None


def _get_nc():
    global _NC_CACHE
    if _NC_CACHE is None:
        _NC_CACHE = build_nc()
    return _NC_CACHE


def make_in_maps(x, W, b, global_A, global_B, local_A, local_B):
    import ml_dtypes

    x = np.asarray(x, dtype=np.float32)
    W = np.asarray(W, dtype=np.float32)
    b = np.asarray(b, dtype=np.float32)
    # W.T tiled [ot][kt][128, 512] -> [8192, 512] so each o-tile is contiguous
    WtT = np.ascontiguousarray(
        W.T.reshape(KT, P, OT, OTILE).transpose(2, 0, 1, 3)
    ).reshape(OT * KT * P, OTILE).astype(np.float16)

    A_cat = np.ascontiguousarray(
        SCALE * np.concatenate([np.asarray(global_A), np.asarray(local_A)],
                               axis=0)).astype(np.float16)
    B_catT = np.ascontiguousarray(
        np.concatenate([np.asarray(global_B).T, np.asarray(local_B).T],
                       axis=0)).astype(np.float16)
    b16 = b.astype(np.float16)

    maps = []
    for i in range(N_CORES):
        xTi = x[i].T  # [DIN, S]
        xT16 = np.ascontiguousarray(xTi[:KT16 * P]).astype(np.float16)
        # DoubleRow pairing [128, 2, S]: [p, j, s] = xT[768 + j*128 + p, s]
        X8 = np.ascontiguousarray(
            xTi[KT16 * P:].reshape(2, P, S).transpose(1, 0, 2)
        ).astype(ml_dtypes.float8_e4m3)
        maps.append({"xT16": xT16, "X8": X8, "WtT": WtT, "b16": b16,
                     "A_cat": A_cat, "B_catT": B_catT})
    return maps


def kernel(x, W, b, global_A, global_B, local_A, local_B):
    nc = _get_nc()
    in_maps = make_in_maps(x, W, b, global_A, global_B, local_A, local_B)
    res = run_bass_kernel_spmd(nc, in_maps, list(range(N_CORES))).results
    return np.stack([res[i]["out"].astype(np.float32)
                     for i in range(N_CORES)], axis=0)
